# revision 25
# baseline (speedup 1.0000x reference)
"""AdaAttN Trainium2 kernel — 8-core SPMD, data-parallel over (batch, query-half).

Each core handles one (batch b, query half): 2048 of the 4096 query positions.
Single-matmul precision strategy (no bf16 two-term splits): the tensor engine
runs fp32r (moving free-dim >= 256) and fp16 matmuls at the same 1 cycle/row
rate as bf16, with ~11-bit-mantissa operand precision (FP22 internal), so:

  F  = f_w @ content_key[b][:, q]   [ck, q]  f32r matmul -> fp16
  G  = g_w @ style_key[b]           [ck, k]  f32r matmul -> fp16
  HT = (h_w @ style[b]).T           [k, c]   f32r matmul -> fp16 (HTF)
  S^T[k, q] = G.T @ F                        fp16 x fp16 matmul (4 MMs/kt)
  P = exp(S^T - 120) -> bf16 (pblk), stored for the whole query block

Consistency discipline for the variance: the bf16 P values are the single
source of truth — the normalizer l = sum_k P (from the same bf16 values),
mean = HTF.T @ P, second = (HTF^2).T @ P with HTF^2 applied as an exact
bf16 pair (h2a stored + h2b derived per tile).  Then second/l - (mean/l)^2
is the exact variance of quantized values under a genuine probability
distribution: nonnegative, no catastrophic-cancellation amplification of
quantization noise.

Pipelining for a gap-free PE stream (HAM stays warm), with elementwise work
spread over three engines (ACT runs Exp only in the steady state — table
reloads are off the critical path):
  pass A per kt: S(kt) MMs (PE), exp->pblk (ACT), l add (GPSIMD), mean MMs
  lagged 6 kt behind; the previous block's post-processing chains are
  emitted at kt==4 (before the first mean group, whose PSUM banks they
  free).  pass B per kt: 8 second-moment MMs, with h2f = HTF^2 (GPSIMD)
  and the bf16 residual h2b (DVE) produced two tiles ahead; next block's
  F projection is emitted inside pass B.  l is partition-reduced on
  GPSIMD (all-reduce) and inverted on DVE.
PSUM: 4 banks ping-pong mean->second (psacc), 4 banks for the S ring and
projections (psmm).  h_b is folded into the final add (variance is
shift-invariant); f_b/g_b are added at F/G PSUM evacuation.

  out = sqrt(relu(second/l - (mean/l)^2)) * mvnorm(content) + mean/l + h_b
"""

import numpy as np

import concourse.bass as bass
import concourse.mybir as mybir
from concourse import bacc
from concourse.bass import ts
from concourse.bass_utils import run_bass_kernel_spmd
from concourse.tile import TileContext
from concourse import bass_isa

F32 = mybir.dt.float32
F32R = mybir.dt.float32r
F16 = mybir.dt.float16
BF16 = mybir.dt.bfloat16
AF = mybir.ActivationFunctionType
ALU = mybir.AluOpType

B, C, HW = 4, 512, 4096  # batch, channels (=key planes), spatial
Q = 2048                 # queries per core (half a batch)
QB = 512                 # query block
QH = 256                 # half-block (DMA/staging granularity)
NBLK = Q // QB           # 4
CC = C // 128            # 4 channel chunks
NKT = HW // 128          # 32 key tiles
LAG = 6                  # mean MMs trail S MMs by this many key tiles
SHIFT = 120.0
EPS = 1e-5


def _build():
    nc = bacc.Bacc("TRN2", target_bir_lowering=False, debug=False)

    ckq = nc.declare_dram_parameter("ckq", [C, Q], F32, isOutput=False)
    sk = nc.declare_dram_parameter("sk", [C, HW], F32, isOutput=False)
    st = nc.declare_dram_parameter("st", [C, HW], F32, isOutput=False)
    ct = nc.declare_dram_parameter("ct", [C, HW], F32, isOutput=False)
    ctq = nc.declare_dram_parameter("ctq", [C, Q], F32, isOutput=False)
    fwT = nc.declare_dram_parameter("fwT", [C, C], F32, isOutput=False)
    gwT = nc.declare_dram_parameter("gwT", [C, C], F32, isOutput=False)
    hwT = nc.declare_dram_parameter("hwT", [C, C], F32, isOutput=False)
    fb = nc.declare_dram_parameter("fb", [C, 1], F32, isOutput=False)
    gb = nc.declare_dram_parameter("gb", [C, 1], F32, isOutput=False)
    hb = nc.declare_dram_parameter("hb", [C, 1], F32, isOutput=False)
    out = nc.declare_dram_parameter("out", [C, Q], F32, isOutput=True)

    # [512, M] dram -> [128, 4, M] (partition = channel-within-chunk)
    def chunked(ap):
        return ap.rearrange("(a p) m -> p a m", p=128)

    with TileContext(nc) as tc:
        with (
            tc.tile_pool(name="const", bufs=1) as const,
            tc.tile_pool(name="stage", bufs=2) as stage,
            tc.tile_pool(name="big", bufs=1) as big,
            tc.tile_pool(name="work", bufs=2) as work,
            tc.tile_pool(name="scratch", bufs=1) as scratch,
            tc.tile_pool(name="psacc", bufs=4, space="PSUM") as psacc,
            tc.tile_pool(name="psmm", bufs=4, space="PSUM") as psmm,
        ):
            # ---------------- constants ----------------
            fwT_sb = const.tile([128, CC, C], F32R)
            nc.sync.dma_start(out=fwT_sb, in_=chunked(fwT.ap()).bitcast(F32R))
            # g_w and h_w are only needed in their (sequential) projection
            # phases — share one SBUF tile, reloading h_w over g_w.
            projw_sb = const.tile([128, CC, C], F32R)
            nc.sync.dma_start(out=projw_sb, in_=chunked(gwT.ap()).bitcast(F32R))
            fb_sb = const.tile([128, CC, 1], F32)
            gb_sb = const.tile([128, CC, 1], F32)
            hb_sb = const.tile([128, CC, 1], F32)
            nc.sync.dma_start(out=fb_sb, in_=chunked(fb.ap()))
            nc.sync.dma_start(out=gb_sb, in_=chunked(gb.ap()))
            nc.sync.dma_start(out=hb_sb, in_=chunked(hb.ap()))
            negshift = const.tile([128, 1], F32)
            nc.vector.memset(negshift, -SHIFT)
            cmean = const.tile([128, CC, 1], F32)
            crstd2 = const.tile([128, CC, 1], F32)

            # ------------- G = g_w @ style_key (f32r) -> fp16 (DVE evac) -------
            Gfp = big.tile([128, CC, HW], F16)
            sk_ch = chunked(sk.ap()).bitcast(F32R)
            for nb in range(HW // 256):
                sk_t = stage.tile([128, CC, 256], F32R, tag="ld4")
                nc.sync.dma_start(out=sk_t, in_=sk_ch[:, :, ts(nb, 256)])
                for co in range(CC):
                    gps = psmm.tile([128, 256], F32, tag="mm")
                    for ci in range(CC):
                        nc.tensor.matmul(
                            gps,
                            projw_sb[:, ci, ts(co, 128)],
                            sk_t[:, ci, :],
                            start=(ci == 0),
                            stop=(ci == CC - 1),
                        )
                    nc.vector.tensor_scalar_add(
                        Gfp[:, co, ts(nb, 256)], gps, gb_sb[:, co, :]
                    )

            # ---------------- main-loop tiles and helpers ----------------
            ckq_ch = chunked(ckq.ap()).bitcast(F32R)
            ctq_ch = chunked(ctq.ap())
            out_ch = chunked(out.ap())
            ct_ch = chunked(ct.ap())
            stats_all = scratch.tile([128, 4, 8, 6], F32, tag="bnstats")
            pblk = big.tile([128, NKT, QB], BF16)

            def emit_F(blk):
                Ffp = work.tile(
                    [128, CC, QB], F16, tag="ffp", name=f"ffp{blk}"
                )
                for hh in range(2):
                    qoff = blk * QB + hh * QH
                    ckq_t = stage.tile(
                        [128, CC, QH], F32R, tag="ld4", name=f"ckq{blk}_{hh}"
                    )
                    nc.sync.dma_start(
                        out=ckq_t, in_=ckq_ch[:, :, qoff : qoff + QH]
                    )
                    hs = slice(hh * QH, (hh + 1) * QH)
                    for co in range(CC):
                        fps = psmm.tile([128, QH], F32, tag="mm")
                        for ci in range(CC):
                            nc.tensor.matmul(
                                fps,
                                fwT_sb[:, ci, ts(co, 128)],
                                ckq_t[:, ci, :],
                                start=(ci == 0),
                                stop=(ci == CC - 1),
                            )
                        nc.vector.tensor_scalar_add(
                            Ffp[:, co, hs], fps, fb_sb[:, co, :]
                        )
                return Ffp

            # F(0) emitted here: its PE work covers the g_w -> h_w reload.
            Ffp = emit_F(0)

            # ------- HT[k, c] = (h_w @ style).T (f32r) -> fp16; h2a = bf16(HT^2)
            # (HT evac on DVE; ACT runs only Square in this phase)
            HTF = big.tile([128, NKT, C], F16)
            h2a = big.tile([128, NKT, C], BF16)
            nc.sync.dma_start(out=projw_sb, in_=chunked(hwT.ap()).bitcast(F32R))
            st_ch = chunked(st.ap()).bitcast(F32R)
            for nb in range(HW // 256):
                st_t = stage.tile([128, CC, 256], F32R, tag="ld4")
                nc.sync.dma_start(out=st_t, in_=st_ch[:, :, ts(nb, 256)])
                for w in range(2):
                    kt = nb * 2 + w
                    hps = psmm.tile([128, 512], F32, tag="mm")
                    for ci in range(CC):
                        nc.tensor.matmul(
                            hps,
                            st_t[:, ci, ts(w, 128)],
                            projw_sb[:, ci, :],
                            start=(ci == 0),
                            stop=(ci == CC - 1),
                        )
                    nc.vector.tensor_copy(HTF[:, kt, :], hps)
                    nc.scalar.activation(
                        h2a[:, kt, :], HTF[:, kt, :], AF.Square, bias=0.0,
                        scale=1.0,
                    )

            def emit_stats_piece(i):
                # piece i: cc = i // 4, quarter = i % 4  -> one DMA + 2 bn_stats
                cc, quart = i // 4, i % 4
                ctp = stage.tile(
                    [128, 4, 256], F32, tag="ld4", name=f"ctp{i}"
                )
                nc.sync.dma_start(
                    out=ctp,
                    in_=ct_ch[:, cc, ts(quart, 1024)].rearrange(
                        "p (a m) -> p a m", a=4
                    ),
                )
                flat = ctp.rearrange("p a m -> p (a m)")
                for g in range(2):
                    nc.vector.bn_stats(
                        out=stats_all[:, cc, quart * 2 + g, :],
                        in_=flat[:, ts(g, 512)],
                    )

            def emit_stats_tail():
                for cc in range(CC):
                    mv = scratch.tile([128, 2], F32, tag="bnmv")
                    nc.vector.bn_aggr(
                        out=mv,
                        in_=stats_all[:, cc, :, :].rearrange("p a b -> p (a b)"),
                    )
                    nc.vector.tensor_copy(cmean[:, cc, :], mv[:, 0:1])
                    tv = scratch.tile([128, 1], F32, tag="bntv")
                    nc.vector.tensor_scalar(
                        out=tv,
                        in0=mv[:, 1:2],
                        scalar1=float(HW) / float(HW - 1),
                        scalar2=EPS,
                        op0=ALU.mult,
                        op1=ALU.add,
                    )
                    nc.vector.reciprocal(crstd2[:, cc, :], tv)

            def emit_e2(sec_ps, rbc, blk):
                # normalize the second moment out of PSUM early: frees the
                # psacc banks for the next block's mean accumulation
                e2s = []
                for cc in range(CC):
                    e2 = scratch.tile(
                        [128, QB], F32, tag="ptmp", bufs=4, name=f"e2_{blk}{cc}"
                    )
                    nc.vector.tensor_mul(e2, sec_ps[cc], rbc)
                    e2s.append(e2)
                return e2s

            def emit_post_rest(e2s, macc, rbc, ct_p0, ct_p1, blk):
                # deprioritized: the scheduler places these during the next
                # pass B, where ACT is idle (Sqrt never interleaves with the
                # Exp stream and its table stays resident per phase)
                for cc in range(CC):
                    mnp_t = work.tile(
                        [128, QB], F32, tag="mnp", name=f"mnpt{blk}{cc}"
                    )
                    nc.gpsimd.tensor_mul(mnp_t, macc[:, cc, :], rbc)
                    msq = work.tile(
                        [128, QB], F32, tag="outb", name=f"msq{blk}{cc}"
                    )
                    nc.gpsimd.tensor_mul(msq, mnp_t, mnp_t)
                    var = work.tile(
                        [128, QB], F32, tag="ptf", name=f"var{blk}{cc}"
                    )
                    nc.gpsimd.tensor_sub(var, e2s[cc], msq)
                    vmx = scratch.tile(
                        [128, QB], F32, tag="po1", bufs=2, name=f"vmx{blk}{cc}"
                    )
                    nc.gpsimd.tensor_scalar_max(vmx, var, 0.0)
                    stdt = work.tile(
                        [128, QB], F32, tag="ptf", name=f"stdt{blk}{cc}"
                    )
                    nc.scalar.activation(
                        stdt, vmx, AF.Sqrt, bias=0.0, scale=crstd2[:, cc, :]
                    )
                    o1 = scratch.tile(
                        [128, QB], F32, tag="po1", bufs=2, name=f"o1_{blk}{cc}"
                    )
                    for hh, ctp in ((0, ct_p0), (1, ct_p1)):
                        nc.vector.scalar_tensor_tensor(
                            out=o1[:, ts(hh, QH)],
                            in0=ctp[:, cc, :],
                            scalar=cmean[:, cc, :],
                            in1=stdt[:, ts(hh, QH)],
                            op0=ALU.subtract,
                            op1=ALU.mult,
                        )
                    out_sb = work.tile(
                        [128, QB], F32, tag="outb", name=f"ob{blk}{cc}"
                    )
                    nc.vector.scalar_tensor_tensor(
                        out=out_sb,
                        in0=mnp_t,
                        scalar=hb_sb[:, cc, :],
                        in1=o1,
                        op0=ALU.add,
                        op1=ALU.add,
                    )
                    nc.sync.dma_start(
                        out=out_ch[:, cc, ts(blk, QB)], in_=out_sb
                    )

            pending_post = None
            for blk in range(NBLK):
                # ---- pass A: S -> P (bf16, stored); mean lags S by LAG kt ----
                mean_ps = [
                    psacc.tile([128, QB], F32, tag="acc", name=f"mean{blk}_{i}")
                    for i in range(CC)
                ]
                l_part = work.tile([128, QB], F32, tag="lpart", bufs=1)

                def emit_mean(kt):
                    for cc in range(CC):
                        nc.tensor.matmul(
                            mean_ps[cc],
                            HTF[:, kt, ts(cc, 128)],
                            pblk[:, kt, :],
                            start=(kt == 0),
                            stop=(kt == NKT - 1),
                        )

                for kt in range(NKT):
                    sps = psmm.tile(
                        [128, QB], F32, tag="mm", name=f"sps{blk}_{kt}"
                    )
                    for ci in range(CC):
                        nc.tensor.matmul(
                            sps,
                            Gfp[:, ci, ts(kt, 128)],
                            Ffp[:, ci, :],
                            start=(ci == 0),
                            stop=(ci == CC - 1),
                        )
                    nc.scalar.activation(
                        pblk[:, kt, :], sps, AF.Exp, bias=negshift, scale=1.0
                    )
                    if kt == 0:
                        nc.vector.tensor_copy(l_part, pblk[:, kt, :])
                    else:
                        nc.vector.tensor_add(l_part, l_part, pblk[:, kt, :])
                    if kt == 4 and pending_post is not None:
                        p_sec, p_macc, p_rbc, p_ct0, p_ct1, p_blk = pending_post
                        e2s = emit_e2(p_sec, p_rbc, p_blk)
                        with tc.high_priority(offset=-400):
                            emit_post_rest(e2s, p_macc, p_rbc, p_ct0, p_ct1, p_blk)
                        pending_post = None
                    if kt >= LAG:
                        emit_mean(kt - LAG)
                    if blk == 0 and kt % 4 == 0:
                        emit_stats_piece(kt // 4)
                for kt in range(NKT - LAG, NKT):
                    emit_mean(kt)

                # l: partition all-reduce (GPSIMD), invert on DVE
                lsum = scratch.tile([128, QB], F32, tag="lsum")
                nc.gpsimd.partition_all_reduce(
                    lsum, l_part, channels=128, reduce_op=bass_isa.ReduceOp.add
                )
                rbc = scratch.tile([128, QB], F32, tag="rbc")
                nc.vector.reciprocal(rbc, lsum)

                # fold mean accumulators to SBUF (DVE) to free PSUM for pass B
                macc = work.tile([128, CC, QB], F32, tag="macc", bufs=1)
                for cc in range(CC):
                    nc.vector.tensor_copy(macc[:, cc, :], mean_ps[cc])

                # ---- pass B: second moment from stored P; h2 split 2 kt ahead
                sec_ps = [
                    psacc.tile([128, QB], F32, tag="acc", name=f"sec{blk}_{i}")
                    for i in range(CC)
                ]
                h2bts = {}

                def emit_h2pipe(kt):
                    h2f = work.tile(
                        [128, C], F32, tag="h2f", bufs=1, name=f"h2f{blk}_{kt}"
                    )
                    nc.vector.tensor_mul(h2f, HTF[:, kt, :], HTF[:, kt, :])
                    h2bt = work.tile(
                        [128, C], BF16, tag="h2bt", bufs=3,
                        name=f"h2bt{blk}_{kt}",
                    )
                    nc.vector.tensor_sub(h2bt, h2f, h2a[:, kt, :])
                    h2bts[kt] = h2bt

                emit_h2pipe(0)
                emit_h2pipe(1)
                for kt in range(NKT):
                    if kt + 2 < NKT:
                        emit_h2pipe(kt + 2)
                    h2bt = h2bts.pop(kt)
                    for cc in range(CC):
                        nc.tensor.matmul(
                            sec_ps[cc],
                            h2a[:, kt, ts(cc, 128)],
                            pblk[:, kt, :],
                            start=(kt == 0),
                            stop=False,
                        )
                        nc.tensor.matmul(
                            sec_ps[cc],
                            h2bt[:, ts(cc, 128)],
                            pblk[:, kt, :],
                            start=False,
                            stop=(kt == NKT - 1),
                        )
                    if kt == 2 and blk + 1 < NBLK:
                        Ffp_next = emit_F(blk + 1)
                    if blk == 0 and kt % 4 == 1:
                        emit_stats_piece(8 + kt // 4)
                if blk == 0:
                    emit_stats_tail()

                ct_p0 = stage.tile(
                    [128, CC, QH], F32, tag="ld4", name=f"ctq{blk}_0"
                )
                nc.sync.dma_start(
                    out=ct_p0, in_=ctq_ch[:, :, blk * QB : blk * QB + QH]
                )
                ct_p1 = stage.tile(
                    [128, CC, QH], F32, tag="ld4", name=f"ctq{blk}_1"
                )
                nc.sync.dma_start(
                    out=ct_p1, in_=ctq_ch[:, :, blk * QB + QH : (blk + 1) * QB]
                )
                pending_post = (sec_ps, macc, rbc, ct_p0, ct_p1, blk)
                if blk + 1 < NBLK:
                    Ffp = Ffp_next
            p_sec, p_macc, p_rbc, p_ct0, p_ct1, p_blk = pending_post
            e2s = emit_e2(p_sec, p_rbc, p_blk)
            emit_post_rest(e2s, p_macc, p_rbc, p_ct0, p_ct1, p_blk)

    nc.compile()
    return nc


_NC_CACHE = []


def kernel(content, style, content_key, style_key, f_w, f_b, g_w, g_b, h_w, h_b):
    if not _NC_CACHE:
        _NC_CACHE.append(_build())
    nc = _NC_CACHE[0]

    c32 = lambda a: np.ascontiguousarray(a, dtype=np.float32)

    fwT = c32(f_w.T)
    gwT = c32(g_w.T)
    hwT = c32(h_w.T)
    fbr = c32(np.asarray(f_b).reshape(C, 1))
    gbr = c32(np.asarray(g_b).reshape(C, 1))
    hbr = c32(np.asarray(h_b).reshape(C, 1))

    in_maps = []
    for core in range(8):
        b, h = core // 2, core % 2
        qsl = slice(h * Q, (h + 1) * Q)
        in_maps.append(
            {
                "ckq": c32(np.asarray(content_key[b]).reshape(C, HW)[:, qsl]),
                "sk": c32(np.asarray(style_key[b]).reshape(C, HW)),
                "st": c32(np.asarray(style[b]).reshape(C, HW)),
                "ct": c32(np.asarray(content[b]).reshape(C, HW)),
                "ctq": c32(np.asarray(content[b]).reshape(C, HW)[:, qsl]),
                "fwT": fwT,
                "gwT": gwT,
                "hwT": hwT,
                "fb": fbr,
                "gb": gbr,
                "hb": hbr,
            }
        )

    res = run_bass_kernel_spmd(nc, in_maps, core_ids=list(range(8)), trace=True)
    kernel.last_exec_time_ns = res.exec_time_ns

    full = np.empty((B, C, HW), dtype=np.float32)
    for core in range(8):
        b, h = core // 2, core % 2
        full[b][:, h * Q : (h + 1) * Q] = res.results[core]["out"]
    return full.reshape(B, C, 64, 64)


kernel.last_exec_time_ns = None


# revision 29
# speedup vs baseline: 1.2543x; 1.2543x over previous
"""AdaAttN Trainium2 kernel — 8-core SPMD, data-parallel over (batch, query-half).

Each core handles one (batch b, query half): 2048 of the 4096 query positions.
Single-matmul precision strategy (no bf16 two-term splits): the tensor engine
runs fp32r (moving free-dim >= 256) and fp16 matmuls at the same 1 cycle/row
rate as bf16, with ~11-bit-mantissa operand precision (FP22 internal), so:

  F  = f_w @ content_key[b][:, q]   [ck, q]  f32r matmul -> fp16
  G  = g_w @ style_key[b]           [ck, k]  f32r matmul -> fp16
  HT = (h_w @ style[b]).T           [k, c]   f32r matmul -> fp16 (HTF)
  S^T[k, q] = G.T @ F                        fp16 x fp16 matmul (4 MMs/kt)
  P = exp(S^T - 120) -> bf16 (pblk), stored for the whole query block

Consistency discipline for the variance: the bf16 P values are the single
source of truth — the normalizer l = sum_k P (from the same bf16 values),
mean = HTF.T @ P, second = (HTF^2).T @ P with HTF^2 applied as an exact
bf16 pair (h2a stored + h2b derived per tile).  Then second/l - (mean/l)^2
is the exact variance of quantized values under a genuine probability
distribution: nonnegative, no catastrophic-cancellation amplification of
quantization noise.

Pipelining for a gap-free PE stream (HAM stays warm), with elementwise work
spread over three engines (ACT runs Exp only in the steady state — table
reloads are off the critical path):
  pass A per kt: S(kt) MMs (PE), exp->pblk (ACT), l add (GPSIMD), mean MMs
  lagged 6 kt behind; the previous block's post-processing chains are
  emitted at kt==4 (before the first mean group, whose PSUM banks they
  free).  pass B per kt: 8 second-moment MMs, with h2f = HTF^2 (GPSIMD)
  and the bf16 residual h2b (DVE) produced two tiles ahead; next block's
  F projection is emitted inside pass B.  l is partition-reduced on
  GPSIMD (all-reduce) and inverted on DVE.
PSUM: 4 banks ping-pong mean->second (psacc), 4 banks for the S ring and
projections (psmm).  h_b is folded into the final add (variance is
shift-invariant); f_b/g_b are added at F/G PSUM evacuation.

  out = sqrt(relu(second/l - (mean/l)^2)) * mvnorm(content) + mean/l + h_b
"""

import numpy as np

import concourse.bass as bass
import concourse.mybir as mybir
from concourse import bacc
from concourse.bass import ts
from concourse.bass_utils import run_bass_kernel_spmd
from concourse.tile import TileContext
from concourse import bass_isa

F32 = mybir.dt.float32
F32R = mybir.dt.float32r
F16 = mybir.dt.float16
BF16 = mybir.dt.bfloat16
AF = mybir.ActivationFunctionType
ALU = mybir.AluOpType

B, C, HW = 4, 512, 4096  # batch, channels (=key planes), spatial
Q = 2048                 # queries per core (half a batch)
QB = 512                 # query block
QH = 256                 # half-block (DMA/staging granularity)
NBLK = Q // QB           # 4
CC = C // 128            # 4 channel chunks
NKT = HW // 128          # 32 key tiles
LAG = 6                  # mean MMs trail S MMs by this many key tiles
SHIFT = 120.0
EPS = 1e-5


def _build():
    nc = bacc.Bacc("TRN2", target_bir_lowering=False, debug=False)

    ckq = nc.declare_dram_parameter("ckq", [C, Q], F32, isOutput=False)
    sk = nc.declare_dram_parameter("sk", [C, HW], F32, isOutput=False)
    st = nc.declare_dram_parameter("st", [C, HW], F32, isOutput=False)
    ct = nc.declare_dram_parameter("ct", [C, HW], F32, isOutput=False)
    ctq = nc.declare_dram_parameter("ctq", [C, Q], F32, isOutput=False)
    fwT = nc.declare_dram_parameter("fwT", [C, C], F32, isOutput=False)
    gwT = nc.declare_dram_parameter("gwT", [C, C], F32, isOutput=False)
    hwT = nc.declare_dram_parameter("hwT", [C, C], F32, isOutput=False)
    fb = nc.declare_dram_parameter("fb", [C, 1], F32, isOutput=False)
    gb = nc.declare_dram_parameter("gb", [C, 1], F32, isOutput=False)
    hb = nc.declare_dram_parameter("hb", [C, 1], F32, isOutput=False)
    out = nc.declare_dram_parameter("out", [C, Q], F32, isOutput=True)

    # [512, M] dram -> [128, 4, M] (partition = channel-within-chunk)
    def chunked(ap):
        return ap.rearrange("(a p) m -> p a m", p=128)

    with TileContext(nc) as tc:
        with (
            tc.tile_pool(name="const", bufs=1) as const,
            tc.tile_pool(name="stage", bufs=3) as stage,
            tc.tile_pool(name="big", bufs=1) as big,
            tc.tile_pool(name="work", bufs=2) as work,
            tc.tile_pool(name="scratch", bufs=1) as scratch,
            tc.tile_pool(name="psacc", bufs=4, space="PSUM") as psacc,
            tc.tile_pool(name="psmm", bufs=4, space="PSUM") as psmm,
        ):
            # ---------------- constants ----------------
            fwT_sb = const.tile([128, CC, C], F32R)
            nc.sync.dma_start(out=fwT_sb, in_=chunked(fwT.ap()).bitcast(F32R))
            # g_w and h_w are only needed in their (sequential) projection
            # phases — share one SBUF tile, reloading h_w over g_w.
            projw_sb = const.tile([128, CC, C], F32R)
            nc.sync.dma_start(out=projw_sb, in_=chunked(gwT.ap()).bitcast(F32R))
            fb_sb = const.tile([128, CC, 1], F32)
            gb_sb = const.tile([128, CC, 1], F32)
            hb_sb = const.tile([128, CC, 1], F32)
            nc.sync.dma_start(out=fb_sb, in_=chunked(fb.ap()))
            nc.sync.dma_start(out=gb_sb, in_=chunked(gb.ap()))
            nc.sync.dma_start(out=hb_sb, in_=chunked(hb.ap()))
            negshift = const.tile([128, 1], F32)
            nc.vector.memset(negshift, -SHIFT)
            cmean = const.tile([128, CC, 1], F32)
            crstd2 = const.tile([128, CC, 1], F32)

            # ------------- G = g_w @ style_key (f32r) -> fp16 (DVE evac) -------
            Gfp = big.tile([128, CC, HW], F16)
            sk_ch = chunked(sk.ap()).bitcast(F32R)
            for nb in range(HW // 256):
                sk_t = stage.tile([128, CC, 256], F32R, tag="ld4")
                nc.sync.dma_start(out=sk_t, in_=sk_ch[:, :, ts(nb, 256)])
                for co in range(CC):
                    gps = psmm.tile([128, 256], F32, tag="mm")
                    for ci in range(CC):
                        nc.tensor.matmul(
                            gps,
                            projw_sb[:, ci, ts(co, 128)],
                            sk_t[:, ci, :],
                            start=(ci == 0),
                            stop=(ci == CC - 1),
                        )
                    nc.vector.tensor_scalar_add(
                        Gfp[:, co, ts(nb, 256)], gps, gb_sb[:, co, :]
                    )

            # ---------------- main-loop tiles and helpers ----------------
            ckq_ch = chunked(ckq.ap()).bitcast(F32R)
            ctq_ch = chunked(ctq.ap())
            out_ch = chunked(out.ap())
            ct_ch = chunked(ct.ap())
            stats_all = scratch.tile([128, 4, 8, 6], F32, tag="bnstats")
            pblk = big.tile([128, NKT, QB], BF16)

            def emit_F(blk):
                Ffp = work.tile(
                    [128, CC, QB], F16, tag="ffp", name=f"ffp{blk}"
                )
                for hh in range(2):
                    qoff = blk * QB + hh * QH
                    ckq_t = stage.tile(
                        [128, CC, QH], F32R, tag="ld4", name=f"ckq{blk}_{hh}"
                    )
                    nc.sync.dma_start(
                        out=ckq_t, in_=ckq_ch[:, :, qoff : qoff + QH]
                    )
                    hs = slice(hh * QH, (hh + 1) * QH)
                    for co in range(CC):
                        fps = psmm.tile([128, QH], F32, tag="mm")
                        for ci in range(CC):
                            nc.tensor.matmul(
                                fps,
                                fwT_sb[:, ci, ts(co, 128)],
                                ckq_t[:, ci, :],
                                start=(ci == 0),
                                stop=(ci == CC - 1),
                            )
                        nc.vector.tensor_scalar_add(
                            Ffp[:, co, hs], fps, fb_sb[:, co, :]
                        )
                return Ffp

            # F(0) emitted here: its PE work covers the g_w -> h_w reload.
            Ffp = emit_F(0)

            # ------- HT[k, c] = (h_w @ style).T (f32r) -> fp16; h2a = bf16(HT^2)
            # (HT evac on DVE; ACT runs only Square in this phase)
            HTF = big.tile([128, NKT, C], F16)
            h2a = big.tile([128, NKT, C], BF16)
            nc.sync.dma_start(out=projw_sb, in_=chunked(hwT.ap()).bitcast(F32R))
            st_ch = chunked(st.ap()).bitcast(F32R)
            for nb in range(HW // 256):
                st_t = stage.tile([128, CC, 256], F32R, tag="ld4")
                nc.sync.dma_start(out=st_t, in_=st_ch[:, :, ts(nb, 256)])
                for w in range(2):
                    kt = nb * 2 + w
                    hps = psmm.tile([128, 512], F32, tag="mm")
                    for ci in range(CC):
                        nc.tensor.matmul(
                            hps,
                            st_t[:, ci, ts(w, 128)],
                            projw_sb[:, ci, :],
                            start=(ci == 0),
                            stop=(ci == CC - 1),
                        )
                    nc.vector.tensor_copy(HTF[:, kt, :], hps)
                    nc.scalar.activation(
                        h2a[:, kt, :], HTF[:, kt, :], AF.Square, bias=0.0,
                        scale=1.0,
                    )

            def emit_stats_piece(i):
                # piece i: cc = i // 4, quarter = i % 4  -> one DMA + 2 bn_stats
                cc, quart = i // 4, i % 4
                ctp = stage.tile(
                    [128, 4, 256], F32, tag="ld4", name=f"ctp{i}"
                )
                nc.sync.dma_start(
                    out=ctp,
                    in_=ct_ch[:, cc, ts(quart, 1024)].rearrange(
                        "p (a m) -> p a m", a=4
                    ),
                )
                flat = ctp.rearrange("p a m -> p (a m)")
                for g in range(2):
                    nc.vector.bn_stats(
                        out=stats_all[:, cc, quart * 2 + g, :],
                        in_=flat[:, ts(g, 512)],
                    )

            def emit_stats_tail():
                for cc in range(CC):
                    mv = scratch.tile([128, 2], F32, tag="bnmv")
                    nc.vector.bn_aggr(
                        out=mv,
                        in_=stats_all[:, cc, :, :].rearrange("p a b -> p (a b)"),
                    )
                    nc.vector.tensor_copy(cmean[:, cc, :], mv[:, 0:1])
                    tv = scratch.tile([128, 1], F32, tag="bntv")
                    nc.vector.tensor_scalar(
                        out=tv,
                        in0=mv[:, 1:2],
                        scalar1=float(HW) / float(HW - 1),
                        scalar2=EPS,
                        op0=ALU.mult,
                        op1=ALU.add,
                    )
                    nc.vector.reciprocal(crstd2[:, cc, :], tv)

            def emit_e2(sec_ps, rbc, blk):
                # normalize the second moment out of PSUM early: frees the
                # psacc banks for the next block's mean accumulation
                e2s = []
                for cc in range(CC):
                    e2 = scratch.tile(
                        [128, QB], F32, tag="ptmp", bufs=4, name=f"e2_{blk}{cc}"
                    )
                    nc.vector.tensor_mul(e2, sec_ps[cc], rbc)
                    e2s.append(e2)
                return e2s

            def emit_post_rest(e2s, macc, rbc, ct_p0, ct_p1, blk):
                # deprioritized: the scheduler places these during the next
                # pass B, where ACT is idle (Sqrt never interleaves with the
                # Exp stream and its table stays resident per phase)
                for cc in range(CC):
                    mnp_t = work.tile(
                        [128, QB], F32, tag="mnp", bufs=1, name=f"mnpt{blk}{cc}"
                    )
                    nc.vector.tensor_mul(mnp_t, macc[:, cc, :], rbc)
                    msq = work.tile(
                        [128, QB], F32, tag="outb", name=f"msq{blk}{cc}"
                    )
                    nc.vector.tensor_mul(msq, mnp_t, mnp_t)
                    var = work.tile(
                        [128, QB], F32, tag="ptf", name=f"var{blk}{cc}"
                    )
                    nc.vector.tensor_sub(var, e2s[cc], msq)
                    vmx = scratch.tile(
                        [128, QB], F32, tag="po1", bufs=2, name=f"vmx{blk}{cc}"
                    )
                    nc.vector.tensor_scalar_max(vmx, var, 0.0)
                    stdt = work.tile(
                        [128, QB], F32, tag="ptf", name=f"stdt{blk}{cc}"
                    )
                    nc.scalar.activation(
                        stdt, vmx, AF.Sqrt, bias=0.0, scale=crstd2[:, cc, :]
                    )
                    o1 = scratch.tile(
                        [128, QB], F32, tag="po1", bufs=2, name=f"o1_{blk}{cc}"
                    )
                    for hh, ctp in ((0, ct_p0), (1, ct_p1)):
                        nc.vector.scalar_tensor_tensor(
                            out=o1[:, ts(hh, QH)],
                            in0=ctp[:, cc, :],
                            scalar=cmean[:, cc, :],
                            in1=stdt[:, ts(hh, QH)],
                            op0=ALU.subtract,
                            op1=ALU.mult,
                        )
                    out_sb = work.tile(
                        [128, QB], F32, tag="outb", name=f"ob{blk}{cc}"
                    )
                    nc.vector.scalar_tensor_tensor(
                        out=out_sb,
                        in0=mnp_t,
                        scalar=hb_sb[:, cc, :],
                        in1=o1,
                        op0=ALU.add,
                        op1=ALU.add,
                    )
                    nc.sync.dma_start(
                        out=out_ch[:, cc, ts(blk, QB)], in_=out_sb
                    )

            pending_post = None
            for blk in range(NBLK):
                # ---- pass A: S -> P (bf16, stored); mean lags S by LAG kt ----
                mean_ps = [
                    psacc.tile([128, QB], F32, tag="acc", name=f"mean{blk}_{i}")
                    for i in range(CC)
                ]
                l_part = work.tile([128, QB], F32, tag="lpart", bufs=1)

                def emit_mean(kt):
                    for cc in range(CC):
                        nc.tensor.matmul(
                            mean_ps[cc],
                            HTF[:, kt, ts(cc, 128)],
                            pblk[:, kt, :],
                            start=(kt == 0),
                            stop=(kt == NKT - 1),
                        )

                for kt in range(NKT):
                    sps = psmm.tile(
                        [128, QB], F32, tag="mm", name=f"sps{blk}_{kt}"
                    )
                    for ci in range(CC):
                        nc.tensor.matmul(
                            sps,
                            Gfp[:, ci, ts(kt, 128)],
                            Ffp[:, ci, :],
                            start=(ci == 0),
                            stop=(ci == CC - 1),
                        )
                    nc.scalar.activation(
                        pblk[:, kt, :], sps, AF.Exp, bias=negshift, scale=1.0
                    )
                    if kt == 0:
                        nc.vector.tensor_copy(l_part, pblk[:, kt, :])
                    else:
                        nc.vector.tensor_add(l_part, l_part, pblk[:, kt, :])
                    if kt == 4 and pending_post is not None:
                        p_sec, p_macc, p_rbc, p_ct0, p_ct1, p_blk = pending_post
                        e2s = emit_e2(p_sec, p_rbc, p_blk)
                        emit_post_rest(e2s, p_macc, p_rbc, p_ct0, p_ct1, p_blk)
                        pending_post = None
                    if kt >= LAG:
                        emit_mean(kt - LAG)
                    if blk == 0 and kt % 4 == 0:
                        emit_stats_piece(kt // 4)
                for kt in range(NKT - LAG, NKT):
                    emit_mean(kt)

                # l: partition all-reduce (GPSIMD), invert on DVE
                lsum = scratch.tile([128, QB], F32, tag="lsum")
                nc.gpsimd.partition_all_reduce(
                    lsum, l_part, channels=128, reduce_op=bass_isa.ReduceOp.add
                )
                rbc = scratch.tile([128, QB], F32, tag="rbc")
                nc.vector.reciprocal(rbc, lsum)

                # fold mean accumulators to SBUF (DVE) to free PSUM for pass B
                macc = work.tile([128, CC, QB], F32, tag="macc", bufs=1)
                for cc in range(CC):
                    nc.vector.tensor_copy(macc[:, cc, :], mean_ps[cc])

                # ---- pass B: second moment from stored P; h2 split 2 kt ahead
                sec_ps = [
                    psacc.tile([128, QB], F32, tag="acc", name=f"sec{blk}_{i}")
                    for i in range(CC)
                ]
                h2bts = {}

                def emit_h2pipe(kt):
                    h2f = work.tile(
                        [128, C], F32, tag="h2f", bufs=1, name=f"h2f{blk}_{kt}"
                    )
                    nc.vector.tensor_mul(h2f, HTF[:, kt, :], HTF[:, kt, :])
                    h2bt = work.tile(
                        [128, C], BF16, tag="h2bt", bufs=3,
                        name=f"h2bt{blk}_{kt}",
                    )
                    nc.vector.tensor_sub(h2bt, h2f, h2a[:, kt, :])
                    h2bts[kt] = h2bt

                emit_h2pipe(0)
                emit_h2pipe(1)
                for kt in range(NKT):
                    if kt + 2 < NKT:
                        emit_h2pipe(kt + 2)
                    h2bt = h2bts.pop(kt)
                    for cc in range(CC):
                        nc.tensor.matmul(
                            sec_ps[cc],
                            h2a[:, kt, ts(cc, 128)],
                            pblk[:, kt, :],
                            start=(kt == 0),
                            stop=False,
                        )
                        nc.tensor.matmul(
                            sec_ps[cc],
                            h2bt[:, ts(cc, 128)],
                            pblk[:, kt, :],
                            start=False,
                            stop=(kt == NKT - 1),
                        )
                    if kt == 2 and blk + 1 < NBLK:
                        Ffp_next = emit_F(blk + 1)
                    if blk == 0 and kt % 4 == 1:
                        emit_stats_piece(8 + kt // 4)
                if blk == 0:
                    emit_stats_tail()

                ct_p0 = stage.tile(
                    [128, CC, QH], F32, tag="ld4", name=f"ctq{blk}_0"
                )
                nc.sync.dma_start(
                    out=ct_p0, in_=ctq_ch[:, :, blk * QB : blk * QB + QH]
                )
                ct_p1 = stage.tile(
                    [128, CC, QH], F32, tag="ld4", name=f"ctq{blk}_1"
                )
                nc.sync.dma_start(
                    out=ct_p1, in_=ctq_ch[:, :, blk * QB + QH : (blk + 1) * QB]
                )
                pending_post = (sec_ps, macc, rbc, ct_p0, ct_p1, blk)
                if blk + 1 < NBLK:
                    Ffp = Ffp_next
            p_sec, p_macc, p_rbc, p_ct0, p_ct1, p_blk = pending_post
            e2s = emit_e2(p_sec, p_rbc, p_blk)
            emit_post_rest(e2s, p_macc, p_rbc, p_ct0, p_ct1, p_blk)

    nc.compile()
    return nc


_NC_CACHE = []


def kernel(content, style, content_key, style_key, f_w, f_b, g_w, g_b, h_w, h_b):
    if not _NC_CACHE:
        _NC_CACHE.append(_build())
    nc = _NC_CACHE[0]

    c32 = lambda a: np.ascontiguousarray(a, dtype=np.float32)

    fwT = c32(f_w.T)
    gwT = c32(g_w.T)
    hwT = c32(h_w.T)
    fbr = c32(np.asarray(f_b).reshape(C, 1))
    gbr = c32(np.asarray(g_b).reshape(C, 1))
    hbr = c32(np.asarray(h_b).reshape(C, 1))

    in_maps = []
    for core in range(8):
        b, h = core // 2, core % 2
        qsl = slice(h * Q, (h + 1) * Q)
        in_maps.append(
            {
                "ckq": c32(np.asarray(content_key[b]).reshape(C, HW)[:, qsl]),
                "sk": c32(np.asarray(style_key[b]).reshape(C, HW)),
                "st": c32(np.asarray(style[b]).reshape(C, HW)),
                "ct": c32(np.asarray(content[b]).reshape(C, HW)),
                "ctq": c32(np.asarray(content[b]).reshape(C, HW)[:, qsl]),
                "fwT": fwT,
                "gwT": gwT,
                "hwT": hwT,
                "fb": fbr,
                "gb": gbr,
                "hb": hbr,
            }
        )

    res = run_bass_kernel_spmd(nc, in_maps, core_ids=list(range(8)), trace=True)
    kernel.last_exec_time_ns = res.exec_time_ns

    full = np.empty((B, C, HW), dtype=np.float32)
    for core in range(8):
        b, h = core // 2, core % 2
        full[b][:, h * Q : (h + 1) * Q] = res.results[core]["out"]
    return full.reshape(B, C, 64, 64)


kernel.last_exec_time_ns = None
